# revision 29
# baseline (speedup 1.0000x reference)
"""Trainium2 Bass kernel for a transformer decoder layer (self-attn + cross-attn + FFN).

Sharding: 8-way tensor parallel over heads for both attentions (2 heads/core),
token-sharded (512 tokens/core) for the wo projections, layernorms and FFN.
Head<->token redistribution is done with three 8-core AllToAll collectives
(self-attn out, cross-attn q, cross-attn out); there are no all-reduces.

All matmuls run in bf16 with fp32 PSUM accumulation. Attention keeps the
[feature, token] (transposed) layout throughout: scoresT uses kT-chunk
stationary x qT moving, probs come out as PT[ki, qi] which feeds attnV
directly with V-natural (+ones column) stationary, producing attn^T and the
softmax denominator in one accumulation group. Scores use a full-128-row
stationary (both heads' KT rows) against zero-padded per-head QT copies so
the PE array sees full-row activity (HAM warmth) at identical cost. Both
heads' scores of one ki-chunk land in one 2-bank PSUM tile and are exp'd by
a single ACT call. Softmax division uses a single-op approximate reciprocal
plus a rank-1 PE broadcast, fused into the stage write.
The cross-attention padding mask is folded into V by zeroing masked key rows
(incl. the ones column), which removes them from output and denominator.
Cross K/V projection streams encT per 512-token stripe and is emitted into
the a2a1/a2aq collective gaps.
"""

import sys

TRN_REPO = "/opt/trn_rl_repo"
if TRN_REPO not in sys.path:
    sys.path.insert(0, TRN_REPO)

import numpy as np
import ml_dtypes

D_MODEL = 1024
N_HEADS = 16
DFF = 4096
B, S = 2, 2048
EPS = 1e-6
DEPTH = D_MODEL // N_HEADS  # 64

NCORES = 8
HPC = N_HEADS // NCORES     # heads per core = 2
TOK = B * S                 # 4096 flattened tokens
TOWN = TOK // NCORES        # 512 tokens per core
KD = D_MODEL // 128         # 8 contraction chunks over d_model
FC = DFF // 128             # 32 chunks over dff
NBT = S // 512              # 4 q-tiles per batch
NBC = S // 128              # 16 ki-chunks per batch

BF = ml_dtypes.bfloat16

_PROG_CACHE = {}


def _build_program(self_blocks, n_ctiles):
    """Emit the SPMD Bass program (same program on all 8 cores).

    self_blocks: dict (t, c) -> 'full' | ('tile', idx) for allowed self-attn
                 blocks (skipped blocks absent), shared by both batches.
    n_ctiles:    number of unique partial-mask tiles in the `cmask` input.
    """
    import concourse.bacc as bacc
    import concourse.mybir as mybir
    from concourse import tile

    F32 = mybir.dt.float32
    F32R = mybir.dt.float32r
    BF16 = mybir.dt.bfloat16
    I16 = mybir.dt.int16
    EXP = mybir.ActivationFunctionType.Exp
    SQRTF = mybir.ActivationFunctionType.Sqrt
    ADD = mybir.AluOpType.add
    MULT = mybir.AluOpType.mult
    SUB = mybir.AluOpType.subtract
    BYPASS = mybir.AluOpType.bypass

    nc = bacc.Bacc("TRN2", target_bir_lowering=False, debug=False,
                   num_devices=NCORES)

    def din(name, shape, dt=BF16):
        return nc.dram_tensor(name, shape, dt, kind="ExternalInput")

    xT_d = din("xT", [D_MODEL, TOK])
    encT_d = din("encT", [D_MODEL, TOK])
    xown_d = din("x_own", [TOWN, D_MODEL], F32)
    wq1_d = din("wq1", [128, KD * 128])
    wk1_d = din("wk1", [128, KD * 128])
    wv1_d = din("wv1", [128, KD * 128])
    bq1_d = din("bq1", [128, 1], F32)
    bk1_d = din("bk1", [128, 1], F32)
    bv1_d = din("bv1", [128, 1], F32)
    wo1_d = din("wo1", [128, KD * 1024])
    bo1_d = din("bo1", [128, 1024], F32)   # pre-broadcast
    wq2_d = din("wq2", [128, KD * KD * 128])
    bq2_d = din("bq2", [128, KD], F32)
    wk2_d = din("wk2", [128, KD * 128])
    wv2_d = din("wv2", [128, KD * 128])
    bk2_d = din("bk2", [128, 1], F32)
    bv2_d = din("bv2", [128, 1], F32)
    wo2_d = din("wo2", [128, KD * 1024])
    bo2_d = din("bo2", [128, 1024], F32)   # pre-broadcast
    w1_d = din("w1", [128, FC * KD * 128])
    b1_d = din("b1", [128, FC], F32)       # per-partition per-chunk
    w2_d = din("w2", [128, FC * 1024])
    b2_d = din("b2", [128, 1024], F32)     # pre-broadcast
    ident_d = din("ident", [128, 128])
    cm_d = din("cmask", [128, max(n_ctiles, 1) * 512])
    mb_d = din("mbias", [128, B * NBC], F32)
    bsel_d = din("bsel", [16, KD * 128])
    out_d = nc.dram_tensor("out", [TOWN, D_MODEL], F32, kind="ExternalOutput")

    CROSS_BLOCKS = {(t, c): 'full' for t in range(NBT) for c in range(NBC)}
    GROUPS = [list(range(NCORES))]

    with tile.TileContext(nc) as tc:
      with tc.tile_pool(name="const", bufs=1) as constp, \
           tc.tile_pool(name="fbuf", bufs=1) as fbuf, \
           tc.tile_pool(name="lnsmall", bufs=2) as lns, \
           tc.tile_pool(name="dram", bufs=1, space="DRAM") as dram, \
           tc.tile_pool(name="ps_aux", bufs=2, space="PSUM") as ps_aux:

        # ---- constants ----
        ones65 = constp.tile([1, 65], F32)
        nc.vector.memset(ones65[:], 1.0)
        ident = constp.tile([128, 128], BF16)
        nc.sync.dma_start(out=ident[:], in_=ident_d[:])
        cm = constp.tile([128, max(n_ctiles, 1) * 512], BF16)
        nc.sync.dma_start(out=cm[:], in_=cm_d[:])
        mb = constp.tile([128, B * NBC], F32)
        nc.sync.dma_start(out=mb[:], in_=mb_d[:])
        bsel = constp.tile([16, KD * 128], BF16)
        nc.sync.dma_start(out=bsel[:], in_=bsel_d[:])

        # ---- persistent activations ----
        out1 = fbuf.tile([128, 4 * 1024], F32, tag="out1")
        outT_a = fbuf.tile([128, KD * 512], BF16, tag="outT")  # out1T

        # ---- a2a dram buffers ----
        bar_in = dram.tile([NCORES, 16], BF16)
        bar_out = dram.tile([NCORES, 16], BF16)
        HT = TOWN // B   # 256 own tokens per batch half
        a2a1_in = [dram.tile([NCORES * 130, HT], BF16, name=f"a2a1i{b}")
                   for b in range(B)]
        a2a1_out = [dram.tile([NCORES * 130, HT], BF16, name=f"a2a1o{b}")
                    for b in range(B)]
        a2aq_in = [dram.tile([NCORES * 128, HT], BF16, name=f"a2aqi{b}")
                   for b in range(B)]
        a2aq_out = [dram.tile([NCORES * 128, HT], BF16, name=f"a2aqo{b}")
                    for b in range(B)]
        a2a2_in = [dram.tile([NCORES * 130, HT], BF16, name=f"a2a2i{b}")
                   for b in range(B)]
        a2a2_out = [dram.tile([NCORES * 130, HT], BF16, name=f"a2a2o{b}")
                    for b in range(B)]

        # startup barrier: absorb cross-core launch skew here (overlapped
        # with the initial input DMAs) instead of inside the first real a2a
        nc.sync.dma_start(out=bar_in[:], in_=ident[0:NCORES, 0:16])
        nc.gpsimd.collective_compute(
            "AllToAll", mybir.AluOpType.bypass, replica_groups=GROUPS,
            ins=[bar_in.opt()], outs=[bar_out.opt()])

        # ---------------- shared helpers ----------------
        def vaug_chunk_tr(vT_sb, vaug_sb, cg, key_mask=None):
            # PE-transpose V chunk cg ([128 (h,d), 128 tok] -> [128 tok,
            # (h,d)]) and scatter into vaug's per-head 65-column groups;
            # key_mask zeroes dropped keys (incl. the ones column).
            b, c = cg // NBC, cg % NBC
            ptr = ps_aux.tile([128, 128], BF16, tag="psaux", name="ptr")
            nc.tensor.transpose(ptr[:], vT_sb[:, 128 * cg:128 * (cg + 1)],
                                ident[:])
            dst = vaug_sb.rearrange("p (h b c d) -> p h b c d",
                                    h=HPC, b=B, c=NBC)
            nc.vector.tensor_copy(dst[:, :, b, c, 0:64],
                                  ptr.rearrange("p (h d) -> p h d", h=HPC))
            if key_mask is not None:
                for h in range(HPC):
                    base = 65 * (NBC * (B * h + b) + c)
                    sl = vaug_sb[:, base:base + 65]
                    nc.vector.tensor_scalar_mul(
                        sl, sl, key_mask[:, NBC * b + c:NBC * b + c + 1])

        def make_vaug(vT_sb, vaug_sb, key_mask=None):
            nc.vector.memset(vaug_sb[:], 1.0)
            for cg in range(TOK // 128):
                vaug_chunk_tr(vT_sb, vaug_sb, cg, key_mask)

        def vaug_slice(vaug_sb, h, b, c):
            base = 65 * (NBC * (B * h + b) + c)
            return vaug_sb[:, base:base + 65]

        EXPA = 0.125 * 1.4426950408889634 * 128.0   # Schraudolph mult
        EXPB = 16250.5                               # 127*128 - 5.5 centering

        def attention(pools, QTz, KT_sb, vaug_sb, stage_sb, blocks,
                      half_cb=None):
            # Software-pipelined attention, one ki-chunk per unit.
            # scores: full-128-row stationary (both heads' KT rows) x
            # zero-padded per-head QT -> both heads' scores in one 2-bank
            # psum; ONE exp per unit; attnV of the previous unit emitted
            # after this unit's scores; softmax division delayed one unit.
            ps_s, ps_o, ptp, smalls = pools
            units = []
            for b in range(B):
                for t in range(NBT):
                    clist = [c for c in range(NBC) if (t, c) in blocks]
                    for i, c in enumerate(clist):
                        units.append((b, t, c, i == 0, i == len(clist) - 1))

            po = {}          # live accumulation psums, per head
            pending = []     # [(unit, pts)] -- attnV delayed 2 units so the
                             # exps always complete well before the PE (in
                             # order) reaches the attnV that consumes them

            def emit_attnv(unit, pts):
                b, t, c, isfirst, islast = unit
                if isfirst:
                    for h in range(HPC):
                        po[h] = ps_o.tile([65, 512], F32, tag=f"po{h}",
                                          name=f"po{h}")
                kind = blocks[(t, c)]
                for h in range(HPC):
                    rhs = pts[h]
                    if kind != 'full':
                        idx = kind[1]
                        nc.vector.tensor_tensor(
                            rhs, rhs, cm[:, 512 * idx:512 * (idx + 1)],
                            op=MULT)
                    nc.tensor.matmul(
                        po[h][:], lhsT=vaug_slice(vaug_sb, h, b, c),
                        rhs=rhs, start=isfirst, stop=islast)
                if islast:
                    emit_division(b, t, dict(po))

            def emit_division(b, t, po_bt):
                # Ship the UNNORMALIZED output plus the denominator row to
                # the stage buffer; normalization happens post-a2a in the wo
                # phase (one approx-reciprocal + rank-1 broadcast per core).
                for h in range(HPC):
                    nc.vector.tensor_copy(
                        stage_sb[:, h * TOK + 2048 * b + 512 * t:
                                 h * TOK + 2048 * b + 512 * t + 512],
                        po_bt[h][:])

            prev_b = 0
            for unit in units:
                b, t, c, isfirst, islast = unit
                if b != prev_b:
                    # flush the pipeline and hand batch 0's outputs to the
                    # caller (fires the first half-collective mid-attention)
                    for p_ in pending:
                        emit_attnv(*p_)
                    pending = []
                    if half_cb is not None:
                        half_cb()
                    prev_b = b
                ps = ps_s.tile([128, 1024], F32, tag="ps4", name="ps4")
                for h in range(HPC):
                    nc.tensor.matmul(
                        ps[:, 512 * h:512 * (h + 1)],
                        lhsT=KT_sb[:, 2048 * b + 128 * c:
                                   2048 * b + 128 * c + 128],
                        rhs=QTz[h][:, 2048 * b + 512 * t:
                                   2048 * b + 512 * t + 512],
                        start=True, stop=True)
                # softmax exp split across engines: h0 exact on ACT, h1
                # via the int16 Schraudolph bit-trick on the (otherwise
                # idle) DVE -- bf16 bits = int16(score*EXPA + EXPB).
                ptA = ptp.tile([128, 512], BF16, tag="ptA", name="ptA")
                nc.scalar.activation(ptA[:], ps[:, 0:512], EXP, scale=0.125)
                ptD = ptp.tile([128, 512], I16, tag="ptD", name="ptD")
                nc.vector.tensor_scalar(ptD[:], ps[:, 512:1024], EXPA, EXPB,
                                        op0=MULT, op1=ADD)
                if len(pending) >= 2:
                    emit_attnv(*pending.pop(0))
                pending.append((unit, (ptA[:], ptD[:].bitcast(BF16))))
            for p_ in pending:
                emit_attnv(*p_)

        def stage_to_a2a(stage_sb, a2a_in_t, b):
            # ship batch b's half: stage cols (h, b, 8 j-subtiles of 256)
            for h in range(HPC):
                o = a2a_in_t.rearrange("(j g r) s -> r j g s", j=NCORES,
                                       g=HPC)
                nc.sync.dma_start(
                    out=o[:, :, h],
                    in_=stage_sb.rearrange("r (h b j s) -> r h b j s",
                                           h=HPC, b=B, j=NCORES)[:, h, b])

        def a2a(in_t, out_t):
            nc.gpsimd.collective_compute(
                "AllToAll", mybir.AluOpType.bypass, replica_groups=GROUPS,
                ins=[in_t.opt()], outs=[out_t.opt()])

        def ln_one_m(pool, pres, m, outf_m):
            # Per-token-block layernorm: stats -> sqrt(var+eps) on ACT
            # (one sqrt table set per phase, no Ln/Exp set thrash) -> DVE
            # approximate reciprocal -> fused (x-mu)*rstd apply. Fully
            # pipelineable against the surrounding matmuls.
            bnst = pool.tile([128, 12], F32, tag="bnst")
            nc.vector.bn_stats(bnst[:, 0:6],
                               pres[:, 1024 * m:1024 * m + 512])
            nc.vector.bn_stats(bnst[:, 6:12],
                               pres[:, 1024 * m + 512:1024 * (m + 1)])
            stats = pool.tile([128, 2], F32, tag="stats")
            nc.vector.bn_aggr(stats[:], bnst[:])
            veps = pool.tile([128, 1], F32, tag="veps")
            nc.vector.tensor_scalar_add(veps[:], stats[:, 1:2], EPS)
            sd = pool.tile([128, 1], F32, tag="sd")
            nc.scalar.activation(sd[:], veps[:], SQRTF)
            rstd = pool.tile([128, 1], F32, tag="rstd")
            nc.vector.reciprocal_approx_fast(out=rstd[:], in_=sd[:])
            nc.vector.tensor_scalar(
                outf_m, pres[:, 1024 * m:1024 * (m + 1)],
                stats[:, 0:1], rstd[:], op0=SUB, op1=MULT)

        def normalize_at(pool, at_sb, a2a_out_t, hb):
            # dn[h, s] = denominator of head h for own-token s (half hb)
            dn = pool.tile([16, HT], BF16, tag=f"dn{hb}", name=f"dn{hb}")
            for g in range(HPC):
                nc.sync.dma_start(
                    out=dn.rearrange("(dc g) s -> g dc s", g=HPC)[g],
                    in_=a2a_out_t.rearrange("(dc g r) s -> g r dc s",
                                            dc=KD, g=HPC)[g, 64])
            dnf = pool.tile([16, HT], F32, tag=f"dnf{hb}", name=f"dnf{hb}")
            nc.vector.tensor_copy(dnf[:], dn[:])
            dnr = pool.tile([16, HT], F32, tag=f"dnr{hb}", name=f"dnr{hb}")
            nc.vector.reciprocal_approx_fast(out=dnr[:], in_=dnf[:])
            dnb = pool.tile([16, HT], BF16, tag=f"dnb{hb}", name=f"dnb{hb}")
            nc.vector.tensor_copy(dnb[:], dnr[:])
            atv = at_sb.rearrange("p (dc b s) -> p dc b s", dc=KD, b=B)
            for dc in range(KD):
                rb = ps_aux.tile([128, HT], F32, tag="psaux", name="rb")
                nc.tensor.matmul(rb[:], lhsT=bsel[:, 128 * dc:128 * (dc + 1)],
                                 rhs=dnb[:], start=True, stop=True)
                nc.vector.tensor_tensor(
                    atv[:, dc, hb], atv[:, dc, hb], rb[:], op=MULT)

        def wo_ln_block(pool1, pool, pstr, at_sb, wo_sb, bo_sb, resid_of,
                        outf, outT_sb, mid_cb=None):
            # outf[:, m*1024: ...] = LN(resid + at^T @ wo + bo), per m-tile.
            # bo_sb is pre-broadcast [128, 1024]; the output transpose runs
            # on the (otherwise idle) PE via the identity trick rather than
            # the serializing DMA-transpose path. LN is batched over all 4
            # m-tiles to avoid ACT table-set thrash.
            pres = pool1.tile([128, 4 * 1024], F32, tag="pres")
            inplace = outf is None
            if inplace:
                outf = pres
            for m in range(4):
                if m == 2 and mid_cb is not None:
                    mid_cb()
                resid = resid_of(m)
                for eh in range(2):
                    pw = ps_aux.tile([128, 512], F32, tag="psaux", name="pw")
                    for dc in range(KD):
                        nc.tensor.matmul(
                            pw[:],
                            lhsT=at_sb[:, 512 * dc + 128 * m:
                                       512 * dc + 128 * m + 128],
                            rhs=wo_sb[:, 1024 * dc + 512 * eh:
                                      1024 * dc + 512 * eh + 512],
                            start=(dc == 0), stop=(dc == KD - 1))
                    nc.vector.tensor_tensor(
                        pres[:, 1024 * m + 512 * eh:
                             1024 * m + 512 * (eh + 1)], pw[:],
                        resid[:, 512 * eh:512 * (eh + 1)], op=ADD)
                    nc.vector.tensor_tensor(
                        pres[:, 1024 * m + 512 * eh:
                             1024 * m + 512 * (eh + 1)],
                        pres[:, 1024 * m + 512 * eh:
                             1024 * m + 512 * (eh + 1)],
                        bo_sb[:, 512 * eh:512 * (eh + 1)], op=ADD)
            outfv = outf.rearrange("p (m e) -> p m e", m=4)
            for m in range(4):
                ln_one_m(pool, pres, m, outfv[:, m])
                # bf16 copy + PE transpose into outT
                obf = pool.tile([128, 1024], BF16, tag="obf")
                nc.vector.tensor_copy(obf[:], outfv[:, m])
                for j in range(KD):
                    ptr = pstr.tile([128, 128], BF16, tag="ptr")
                    nc.tensor.transpose(ptr[:], obf[:, 128 * j:128 * (j + 1)],
                                        ident[:])
                    nc.vector.tensor_copy(
                        outT_sb[:, 512 * j + 128 * m:512 * j + 128 * m + 128],
                        ptr[:])
            return outf

        # p3keep: cross-attn K/V/Q buffers that must survive into attn2
        with tc.tile_pool(name="p3keep", bufs=1) as p3k:
            KT2 = p3k.tile([128, TOK], BF16, tag="KT2")
            vaug2 = p3k.tile([128, HPC * B * NBC * 65], BF16, tag="vaug2")
            vT2 = p3k.tile([128, TOK], BF16, tag="vT2")
            wk2 = p3k.tile([128, KD * 128], BF16, tag="wk2")
            wv2 = p3k.tile([128, KD * 128], BF16, tag="wv2")
            bk2 = p3k.tile([128, 1], F32, tag="bk2")
            bv2 = p3k.tile([128, 1], F32, tag="bv2")

            # ============= phases 1-2: self attention =====================
            pe3s = tc.alloc_tile_pool(name="pencT", bufs=3)
            enc_pre = []

            def prefetch_enc(n):
                for j in range(len(enc_pre), n):
                    et = pe3s.tile([128, KD * 512], BF16, tag="et",
                                   name=f"etp{j}")
                    nc.sync.dma_start(
                        out=et.rearrange("p (k s) -> p k s", k=KD),
                        in_=encT_d.rearrange(
                            "(k p) t -> p k t",
                            p=128)[:, :, 512 * j:512 * (j + 1)])
                    enc_pre.append(et)
            with tc.tile_pool(name="pact1", bufs=1) as pact1, \
                 tc.tile_pool(name="p12s", bufs=2) as p12s, \
                 tc.tile_pool(name="pt12", bufs=4) as pt12:
                KT = pact1.tile([128, TOK], BF16, tag="KT")
                # vT shares the (larger) stage slot — it dies before stage1
                # is written.
                vT1 = pact1.tile([128, TOK], BF16, tag="stage", name="vT1")
                vaug1 = pact1.tile([128, HPC * B * NBC * 65], BF16,
                                   tag="vaug")
                QTz0 = pact1.tile([128, TOK], BF16, tag="QTz0")
                QTz1 = pact1.tile([128, TOK], BF16, tag="QTz1")
                nc.vector.memset(QTz0[64:128, :], 0.0)
                nc.vector.memset(QTz1[0:64, :], 0.0)

                with tc.tile_pool(name="pxw", bufs=1) as pxw, \
                     tc.tile_pool(name="pxs", bufs=2) as pxs:
                    wq1 = pxw.tile([128, KD * 128], BF16, tag="wq1")
                    wk1 = pxw.tile([128, KD * 128], BF16, tag="wk1")
                    wv1 = pxw.tile([128, KD * 128], BF16, tag="wv1")
                    nc.sync.dma_start(out=wq1[:], in_=wq1_d[:])
                    nc.sync.dma_start(out=wk1[:], in_=wk1_d[:])
                    nc.sync.dma_start(out=wv1[:], in_=wv1_d[:])
                    bq1 = pxw.tile([128, 1], F32, tag="bq1")
                    bk1 = pxw.tile([128, 1], F32, tag="bk1")
                    bv1 = pxw.tile([128, 1], F32, tag="bv1")
                    nc.sync.dma_start(out=bq1[:], in_=bq1_d[:])
                    nc.sync.dma_start(out=bk1[:], in_=bk1_d[:])
                    nc.sync.dma_start(out=bv1[:], in_=bv1_d[:])
                    nc.sync.dma_start(out=bk2[:], in_=bk2_d[:])
                    nc.sync.dma_start(out=bv2[:], in_=bv2_d[:])
                    nc.sync.dma_start(out=wk2[:], in_=wk2_d[:])
                    nc.sync.dma_start(out=wv2[:], in_=wv2_d[:])
                    nc.vector.memset(vaug1[:], 1.0)

                    def to_qtz(ps, j):
                        nc.vector.tensor_scalar_add(
                            QTz0[0:64, 512 * j:512 * (j + 1)], ps[0:64, :],
                            bq1[0:64, :])
                        nc.vector.tensor_scalar_add(
                            QTz1[64:128, 512 * j:512 * (j + 1)],
                            ps[64:128, :], bq1[64:128, :])

                    # xT streamed per 512-token stripe: the v/q/k projections
                    # for one stripe start after ~1/8 of the input load
                    xT_dv = xT_d.rearrange("(k p) t -> p k t", p=128)
                    for j in range(TOK // 512):
                        xs = pxs.tile([128, KD * 512], BF16, tag="xs")
                        xsv = xs.rearrange("p (k s) -> p k s", k=KD)
                        nc.sync.dma_start(
                            out=xsv[:],
                            in_=xT_dv[:, :, 512 * j:512 * (j + 1)])

                        def proj_t(dst_of):
                            w_sb = dst_of[0]
                            ps = ps_aux.tile([128, 512], F32, tag="psaux",
                                             name="psp")
                            for k in range(KD):
                                nc.tensor.matmul(
                                    ps[:],
                                    lhsT=w_sb[:, 128 * k:128 * (k + 1)],
                                    rhs=xsv[:, k], start=(k == 0),
                                    stop=(k == KD - 1))
                            dst_of[1](ps)

                        def to_full(dst, bias):
                            def f(ps):
                                nc.vector.tensor_scalar_add(
                                    dst[:, 512 * j:512 * (j + 1)], ps[:],
                                    bias[:])
                            return f

                        proj_t((wv1, to_full(vT1, bv1)))
                        for cg in range(4 * j, 4 * j + 4):
                            vaug_chunk_tr(vT1, vaug1, cg)
                        proj_t((wq1, lambda ps: to_qtz(ps, j)))
                        proj_t((wk1, to_full(KT, bk1)))
                # pxw/pxs closed
                prefetch_enc(2)

                stage1 = pact1.tile([65, HPC * TOK], BF16, tag="stage",
                                    name="stage1")

                def attn1_half():
                    stage_to_a2a(stage1, a2a1_in[0], 0)
                    a2a(a2a1_in[0], a2a1_out[0])

                with tc.tile_pool(name="ps_s1", bufs=2,
                                  space="PSUM") as ps_s1, \
                     tc.tile_pool(name="ps_o1", bufs=1,
                                  space="PSUM") as ps_o1:
                    attention((ps_s1, ps_o1, pt12, p12s),
                              (QTz0, QTz1), KT, vaug1, stage1, self_blocks,
                              half_cb=attn1_half)
                stage_to_a2a(stage1, a2a1_in[1], 1)
            # pact1 closed (attn1 buffers free)

            a2a(a2a1_in[1], a2a1_out[1])

            # ===== phase 3: cross K/V projection =========================
            # encT streamed per 512-token stripe (first two prefetched
            # during attn1); the first half is emitted between wo1's two
            # m-halves (fills the a2a1B wait), the rest after the a2aq
            # doorbells (fills that gap).
            def emit_cross_stripes(j0, j1):
                for j in range(j0, j1):
                    if j < len(enc_pre):
                        et = enc_pre[j]
                    else:
                        et = pe3s.tile([128, KD * 512], BF16, tag="et",
                                       name=f"et{j}")
                        nc.sync.dma_start(
                            out=et.rearrange("p (k s) -> p k s", k=KD),
                            in_=encT_d.rearrange(
                                "(k p) t -> p k t",
                                p=128)[:, :, 512 * j:512 * (j + 1)])
                    etv = et.rearrange("p (k s) -> p k s", k=KD)
                    for w_sb, bias, dst in ((wv2, bv2, vT2), (wk2, bk2, KT2)):
                        ps = ps_aux.tile([128, 512], F32, tag="psaux",
                                         name="psc")
                        for k in range(KD):
                            nc.tensor.matmul(
                                ps[:],
                                lhsT=w_sb[:, 128 * k:128 * (k + 1)],
                                rhs=etv[:, k], start=(k == 0),
                                stop=(k == KD - 1))
                        nc.vector.tensor_scalar_add(
                            dst[:, 512 * j:512 * (j + 1)], ps[:], bias[:])

            # ===== phase 4: wo1 + residual + LN1 + transpose ===============
            with tc.tile_pool(name="p4", bufs=1) as p4, \
                 tc.tile_pool(name="p4s", bufs=2) as p4s:
                at1 = p4.tile([128, KD * 512], BF16, tag="at1")

                def load_at_half(at_sb, srcs, hb):
                    for g in range(HPC):
                        nc.sync.dma_start(
                            out=at_sb.rearrange("p (dc b s) -> p dc b s",
                                                dc=KD, b=B)[
                                64 * g:64 * (g + 1), :, hb],
                            in_=srcs[hb].rearrange(
                                "(dc g r) s -> g r dc s",
                                dc=KD, g=HPC)[g, 0:64])

                load_at_half(at1, a2a1_out, 0)
                normalize_at(p4, at1, a2a1_out[0], 0)
                wo1 = p4.tile([128, KD * 1024], BF16, tag="wo1")
                nc.sync.dma_start(out=wo1[:], in_=wo1_d[:])
                bo1 = p4.tile([128, 1024], F32, tag="bo1")
                nc.sync.dma_start(out=bo1[:], in_=bo1_d[:])
                xown = p4.tile([128, 4 * 1024], F32, tag="xown")
                nc.sync.dma_start(
                    out=xown.rearrange("p (m e) -> p m e", m=4),
                    in_=xown_d.rearrange("(m p) e -> p m e", p=128))
                xownv = xown.rearrange("p (m e) -> p m e", m=4)

                with tc.tile_pool(name="ps_tr4", bufs=2,
                                  space="PSUM") as ps_tr4:
                    def wo1_mid():
                        load_at_half(at1, a2a1_out, 1)
                        emit_cross_stripes(0, 3)
                        normalize_at(p4, at1, a2a1_out[1], 1)

                    wo_ln_block(p4, p4s, ps_tr4, at1, wo1, bo1,
                                lambda m: xownv[:, m], out1, outT_a,
                                mid_cb=wo1_mid)

            emit_cross_stripes(3, 5)

            # ===== phase 5: cross q projection + a2a ========================
            with tc.tile_pool(name="p5", bufs=1) as p5:
                wq2 = p5.tile([128, KD * KD * 128], BF16, tag="wq2")
                nc.sync.dma_start(out=wq2[:], in_=wq2_d[:])
                bq2 = p5.tile([128, KD], F32, tag="bq2")
                nc.sync.dma_start(out=bq2[:], in_=bq2_d[:])
                qt2 = p5.tile([128, KD * 512], BF16, tag="qt2")
                for j in range(KD):
                    pq = ps_aux.tile([128, 512], F32, tag="psaux",
                                     name="pq")
                    for k in range(KD):
                        nc.tensor.matmul(
                            pq[:],
                            lhsT=wq2[:, 1024 * j + 128 * k:
                                     1024 * j + 128 * k + 128],
                            rhs=outT_a[:, 512 * k:512 * (k + 1)],
                            start=(k == 0), stop=(k == KD - 1))
                    nc.vector.tensor_scalar_add(
                        qt2[:, 512 * j:512 * (j + 1)], pq[:],
                        bq2[:, j:j + 1])
                for hb in range(B):
                    nc.sync.dma_start(
                        out=a2aq_in[hb].rearrange("(j p) s -> p j s", p=128),
                        in_=qt2.rearrange("p (j b s) -> p j b s", j=KD,
                                          b=B)[:, :, hb])
            a2a(a2aq_in[0], a2aq_out[0])
            a2a(a2aq_in[1], a2aq_out[1])

            emit_cross_stripes(5, 8)
            pe3s.release()
            # vaug2 build follows the cross projections (needs only vT2)
            make_vaug(vT2, vaug2, key_mask=mb)

            QT2z0 = p3k.tile([128, TOK], BF16, tag="qt2z0", name="QT2z0")
            QT2z1 = p3k.tile([128, TOK], BF16, tag="qt2z1", name="QT2z1")
            nc.vector.memset(QT2z0[64:128, :], 0.0)
            nc.vector.memset(QT2z1[0:64, :], 0.0)
            for hb in range(B):
                aqv = a2aq_out[hb].rearrange("(i p) s -> p i s", p=128)
                z0v = QT2z0.rearrange("p (b i s) -> p b i s", b=B,
                                      i=NCORES)
                z1v = QT2z1.rearrange("p (b i s) -> p b i s", b=B,
                                      i=NCORES)
                nc.sync.dma_start(out=z0v[0:64, hb], in_=aqv[0:64])
                nc.sync.dma_start(out=z1v[64:128, hb], in_=aqv[64:128])

            # ===== phase 6: cross attention -> a2a2 =========================
            with tc.tile_pool(name="p6", bufs=1) as p6, \
                 tc.tile_pool(name="p6s", bufs=2) as p6s, \
                 tc.tile_pool(name="pt6", bufs=4) as pt6, \
                 tc.tile_pool(name="ps_s2", bufs=2, space="PSUM") as ps_s2, \
                 tc.tile_pool(name="ps_o2", bufs=1, space="PSUM") as ps_o2:
                stage2 = p6.tile([65, HPC * TOK], BF16, tag="stage2")

                def attn2_half():
                    stage_to_a2a(stage2, a2a2_in[0], 0)
                    a2a(a2a2_in[0], a2a2_out[0])

                attention((ps_s2, ps_o2, pt6, p6s),
                          (QT2z0, QT2z1), KT2, vaug2, stage2, CROSS_BLOCKS,
                          half_cb=attn2_half)
                stage_to_a2a(stage2, a2a2_in[1], 1)
            a2a(a2a2_in[1], a2a2_out[1])

        # ============ phases 7-8: wo2 + LN2 + FFN + LN3 =====================
        with tc.tile_pool(name="p78", bufs=1) as p78, \
             tc.tile_pool(name="p78s", bufs=2) as p78s, \
             tc.tile_pool(name="w1stream", bufs=3) as w1s_pool:
            at2 = p78.tile([128, KD * 512], BF16, tag="at2")

            def load_at2_half(hb):
                for g in range(HPC):
                    nc.sync.dma_start(
                        out=at2.rearrange("p (dc b s) -> p dc b s",
                                          dc=KD, b=B)[
                            64 * g:64 * (g + 1), :, hb],
                        in_=a2a2_out[hb].rearrange(
                            "(dc g r) s -> g r dc s",
                            dc=KD, g=HPC)[g, 0:64])

            load_at2_half(0)
            normalize_at(p78, at2, a2a2_out[0], 0)
            wo2 = p78.tile([128, KD * 1024], BF16, tag="wo2")
            nc.sync.dma_start(out=wo2[:], in_=wo2_d[:])
            bo2 = p78.tile([128, 1024], F32, tag="bo2")
            nc.sync.dma_start(out=bo2[:], in_=bo2_d[:])
            b1 = p78.tile([128, FC], F32, tag="b1")
            nc.sync.dma_start(out=b1[:], in_=b1_d[:])
            b2 = p78.tile([128, 1024], F32, tag="b2")
            nc.sync.dma_start(out=b2[:], in_=b2_d[:])
            w2 = p78.tile([128, FC * 1024], BF16, tag="w2")

            # out2T reuses the out1T slot (out1T dead after phase 5)
            outT_b = fbuf.tile([128, KD * 512], BF16, tag="outT",
                               name="outT_b")
            out1v = out1.rearrange("p (m e) -> p m e", m=4)
            with tc.tile_pool(name="ps_tr78", bufs=2,
                              space="PSUM") as ps_tr78:
                def wo2_mid():
                    load_at2_half(1)
                    # big FFN w2 weight load AFTER the collective's data is
                    # in flight so the 8MB stream doesn't fight the a2a for
                    # HBM bandwidth
                    nc.sync.dma_start(out=w2[:], in_=w2_d[:])
                    normalize_at(p78, at2, a2a2_out[1], 1)

                out2 = wo_ln_block(p78, p78s, ps_tr78, at2, wo2, bo2,
                                   lambda m: out1v[:, m], None, outT_b,
                                   mid_cb=wo2_mid)

            hT = p78.tile([128, FC * 512], BF16, tag="hT")
            for fc in range(FC):
                w1t = w1s_pool.tile([128, KD * 128], BF16, tag="w1s")
                nc.sync.dma_start(out=w1t[:],
                                  in_=w1_d[:, 1024 * fc:1024 * (fc + 1)])
                ph = ps_aux.tile([128, 512], F32, tag="psaux", name="ph")
                for k in range(KD):
                    nc.tensor.matmul(ph[:],
                                     lhsT=w1t[:, 128 * k:128 * (k + 1)],
                                     rhs=outT_b[:, 512 * k:512 * (k + 1)],
                                     start=(k == 0), stop=(k == KD - 1))
                nc.vector.tensor_scalar(hT[:, 512 * fc:512 * (fc + 1)],
                                        ph[:], b1[:, fc:fc + 1], 0.0,
                                        op0=ADD, op1=mybir.AluOpType.max)

            out2v = out2.rearrange("p (m e) -> p m e", m=4)
            pres2 = p78.tile([128, 4 * 1024], F32, tag="pres2")
            with tc.tile_pool(name="ps_w2", bufs=2, space="PSUM") as ps_w2:
                for m in range(4):
                    # eh inner: both 512-col halves share each hT stationary
                    py = ps_w2.tile([128, 1024], F32, tag="py", name="py")
                    for fc in range(FC):
                        for eh in range(2):
                            nc.tensor.matmul(
                                py[:, 512 * eh:512 * (eh + 1)],
                                lhsT=hT[:, 512 * fc + 128 * m:
                                        512 * fc + 128 * m + 128],
                                rhs=w2[:, 1024 * fc + 512 * eh:
                                       1024 * fc + 512 * eh + 512],
                                start=(fc == 0), stop=(fc == FC - 1))
                    for eh in range(2):
                        nc.vector.tensor_tensor(
                            pres2[:, 1024 * m + 512 * eh:
                                  1024 * m + 512 * (eh + 1)],
                            py[:, 512 * eh:512 * (eh + 1)],
                            out2v[:, m, 512 * eh:512 * (eh + 1)], op=ADD)
                        nc.vector.tensor_tensor(
                            pres2[:, 1024 * m + 512 * eh:
                                  1024 * m + 512 * (eh + 1)],
                            pres2[:, 1024 * m + 512 * eh:
                                  1024 * m + 512 * (eh + 1)],
                            b2[:, 512 * eh:512 * (eh + 1)], op=ADD)

            # final LN applied in-place into pres2, then DMA'd out, per m
            pres2v = pres2.rearrange("p (m e) -> p m e", m=4)
            for m in range(4):
                ln_one_m(p78s, pres2, m, pres2v[:, m])
                nc.sync.dma_start(out=out_d[128 * m:128 * (m + 1), :],
                                  in_=pres2v[:, m])

    nc.compile()
    return nc


def _to_bf(a):
    return np.ascontiguousarray(np.asarray(a, np.float32).astype(BF))


def _rechunk_k(w):
    """[K*128, M] -> [128, K*M] with col k*M + m = w[k*128 + p, m]."""
    K = w.shape[0] // 128
    M = w.shape[1]
    return np.ascontiguousarray(
        w.reshape(K, 128, M).transpose(1, 0, 2).reshape(128, K * M))


def _analyze_self_mask(mask):
    """mask [S, S] (1 = disallowed), orientation [q, k].

    Returns blocks dict (t, c) -> 'full' | ('tile', idx), list of unique
    multiplicative tiles [128, 512] (bf16), for a block grid over one batch.
    Blocks where everything is disallowed are omitted.
    """
    add = np.float32(-1e9) * np.asarray(mask, np.float32)
    mult = np.exp(add.T)  # [k, q] multiplicative
    blocks = {}
    tiles = []
    tile_ids = {}
    for t in range(NBT):
        for c in range(NBC):
            sub = mult[128 * c:128 * (c + 1), 512 * t:512 * (t + 1)]
            if not sub.any():
                continue
            if (sub == 1.0).all():
                blocks[(t, c)] = 'full'
                continue
            key = sub.tobytes()
            if key not in tile_ids:
                tile_ids[key] = len(tiles)
                tiles.append(sub.astype(BF))
            blocks[(t, c)] = ('tile', tile_ids[key])
    return blocks, tiles


def kernel(**inputs):
    from concourse.bass_utils import run_bass_kernel_spmd

    x = np.asarray(inputs["x"], np.float32)
    enc = np.asarray(inputs["enc_output"], np.float32)
    lam = np.asarray(inputs["look_ahead_mask"], np.float32)[0, 0]
    pad = np.asarray(inputs["padding_mask"], np.float32)  # [B,1,1,S]

    self_blocks, ctiles = _analyze_self_mask(lam)
    n_ctiles = len(ctiles)
    key = (tuple(sorted(self_blocks.items())), n_ctiles)
    if key not in _PROG_CACHE:
        _PROG_CACHE[key] = _build_program(self_blocks, n_ctiles)
    nc = _PROG_CACHE[key]

    # ---- shared (core-independent) host prep ----
    xf = x.reshape(TOK, D_MODEL)             # flattened batch-major tokens
    encf = enc.reshape(TOK, D_MODEL)
    xT = _to_bf(xf.T)                        # [1024, 4096]
    encT = _to_bf(encf.T)
    if n_ctiles:
        cmask = np.concatenate(ctiles, axis=1)
    else:
        cmask = np.zeros((128, 512), BF)
    cmask = np.ascontiguousarray(cmask)
    # cross-attn key-keep mask per enc token: [128, B*16], col b*16+c
    mb = np.exp(np.float32(-1e9) * pad[:, 0, 0, :]).reshape(B, NBC, 128)
    mb = np.ascontiguousarray(mb.transpose(2, 0, 1).reshape(128, B * NBC)
                              ).astype(np.float32)

    w1f = np.asarray(inputs["ffn_w1"], np.float32)
    # w1 stationary layout: [128, fc*1024 + k*128 + m] = w1[k*128+p, fc*128+m]
    w1r = w1f.reshape(KD, 128, FC, 128).transpose(1, 2, 0, 3)
    w1r = _to_bf(w1r.reshape(128, FC * KD * 128))
    w2r = _to_bf(_rechunk_k(np.asarray(inputs["ffn_w2"], np.float32)))
    # b1 per-partition per-chunk [128, FC]; b2/bo pre-broadcast [128, 1024]
    b1 = np.ascontiguousarray(
        np.asarray(inputs["ffn_b1"], np.float32).reshape(FC, 128).T)
    b2 = np.ascontiguousarray(np.broadcast_to(
        np.asarray(inputs["ffn_b2"], np.float32)[None, :], (128, 1024)))

    wo1r = _to_bf(_rechunk_k(np.asarray(inputs["mha1_wo"], np.float32)))
    wo2r = _to_bf(_rechunk_k(np.asarray(inputs["mha2_wo"], np.float32)))
    bo1 = np.ascontiguousarray(np.broadcast_to(
        np.asarray(inputs["mha1_bo"], np.float32)[None, :], (128, 1024)))
    bo2 = np.ascontiguousarray(np.broadcast_to(
        np.asarray(inputs["mha2_bo"], np.float32)[None, :], (128, 1024)))
    ident = np.eye(128, dtype=np.float32).astype(BF)
    # bsel[h, dc*128 + p] = 1 where head h owns feature row p of chunk dc
    bsel = np.zeros((16, KD * 128), np.float32)
    for dc in range(KD):
        for p in range(128):
            bsel[2 * dc + p // 64, 128 * dc + p] = 1.0
    bsel = bsel.astype(BF)

    wq2_full = np.asarray(inputs["mha2_wq"], np.float32)
    # wq2 stationary layout: [128, j*1024 + k*128 + m] = wq2[k*128+p, j*128+m]
    wq2r = wq2_full.reshape(KD, 128, KD, 128).transpose(1, 2, 0, 3)
    wq2r = _to_bf(wq2r.reshape(128, KD * KD * 128))
    bq2 = np.asarray(inputs["mha2_bq"], np.float32).reshape(KD, 128)
    bq2 = np.ascontiguousarray(bq2.T).astype(np.float32)  # [128, KD]

    in_maps = []
    for j in range(NCORES):
        hs = slice(128 * j, 128 * (j + 1))       # this core's 2 heads' cols
        m = {
            "xT": xT, "encT": encT,
            "x_own": np.ascontiguousarray(np.concatenate(
                [xf[256 * j:256 * (j + 1)],
                 xf[2048 + 256 * j:2048 + 256 * (j + 1)]], axis=0)),
            "cmask": cmask, "mbias": mb,
            "w1": w1r, "b1": b1, "w2": w2r, "b2": b2,
            "wo1": wo1r, "bo1": bo1, "wo2": wo2r, "bo2": bo2,
            "wq2": wq2r, "bq2": bq2, "ident": ident, "bsel": bsel,
        }
        for pre, name in (("wq1", "mha1_wq"), ("wk1", "mha1_wk"),
                          ("wv1", "mha1_wv"), ("wk2", "mha2_wk"),
                          ("wv2", "mha2_wv")):
            w = np.asarray(inputs[name], np.float32)[:, hs]
            m[pre] = _to_bf(_rechunk_k(w))
        for pre, name in (("bq1", "mha1_bq"), ("bk1", "mha1_bk"),
                          ("bv1", "mha1_bv"), ("bk2", "mha2_bk"),
                          ("bv2", "mha2_bv")):
            bvec = np.asarray(inputs[name], np.float32)[hs]
            m[pre] = np.ascontiguousarray(bvec[:, None])
        in_maps.append(m)

    res = run_bass_kernel_spmd(nc, in_maps, list(range(NCORES)))
    out = np.empty((TOK, D_MODEL), np.float32)
    for j in range(NCORES):
        r = res.results[j]["out"]
        out[256 * j:256 * (j + 1)] = r[0:256]
        out[2048 + 256 * j:2048 + 256 * (j + 1)] = r[256:512]
    return out.reshape(B, S, D_MODEL)


# revision 30
# speedup vs baseline: 1.0013x; 1.0013x over previous
"""Trainium2 Bass kernel for a transformer decoder layer (self-attn + cross-attn + FFN).

Sharding: 8-way tensor parallel over heads for both attentions (2 heads/core),
token-sharded (512 tokens/core) for the wo projections, layernorms and FFN.
Head<->token redistribution is done with three 8-core AllToAll collectives
(self-attn out, cross-attn q, cross-attn out); there are no all-reduces.

All matmuls run in bf16 with fp32 PSUM accumulation. Attention keeps the
[feature, token] (transposed) layout throughout: scoresT uses kT-chunk
stationary x qT moving, probs come out as PT[ki, qi] which feeds attnV
directly with V-natural (+ones column) stationary, producing attn^T and the
softmax denominator in one accumulation group. Scores use a full-128-row
stationary (both heads' KT rows) against zero-padded per-head QT copies so
the PE array sees full-row activity (HAM warmth) at identical cost. Both
heads' scores of one ki-chunk land in one 2-bank PSUM tile and are exp'd by
a single ACT call. Softmax division uses a single-op approximate reciprocal
plus a rank-1 PE broadcast, fused into the stage write.
The cross-attention padding mask is folded into V by zeroing masked key rows
(incl. the ones column), which removes them from output and denominator.
Cross K/V projection streams encT per 512-token stripe and is emitted into
the a2a1/a2aq collective gaps.
"""

import sys

TRN_REPO = "/opt/trn_rl_repo"
if TRN_REPO not in sys.path:
    sys.path.insert(0, TRN_REPO)

import numpy as np
import ml_dtypes

D_MODEL = 1024
N_HEADS = 16
DFF = 4096
B, S = 2, 2048
EPS = 1e-6
DEPTH = D_MODEL // N_HEADS  # 64

NCORES = 8
HPC = N_HEADS // NCORES     # heads per core = 2
TOK = B * S                 # 4096 flattened tokens
TOWN = TOK // NCORES        # 512 tokens per core
KD = D_MODEL // 128         # 8 contraction chunks over d_model
FC = DFF // 128             # 32 chunks over dff
NBT = S // 512              # 4 q-tiles per batch
NBC = S // 128              # 16 ki-chunks per batch

BF = ml_dtypes.bfloat16

_PROG_CACHE = {}


def _build_program(self_blocks, n_ctiles):
    """Emit the SPMD Bass program (same program on all 8 cores).

    self_blocks: dict (t, c) -> 'full' | ('tile', idx) for allowed self-attn
                 blocks (skipped blocks absent), shared by both batches.
    n_ctiles:    number of unique partial-mask tiles in the `cmask` input.
    """
    import concourse.bacc as bacc
    import concourse.mybir as mybir
    from concourse import tile

    F32 = mybir.dt.float32
    F32R = mybir.dt.float32r
    BF16 = mybir.dt.bfloat16
    I16 = mybir.dt.int16
    EXP = mybir.ActivationFunctionType.Exp
    SQRTF = mybir.ActivationFunctionType.Sqrt
    ADD = mybir.AluOpType.add
    MULT = mybir.AluOpType.mult
    SUB = mybir.AluOpType.subtract
    BYPASS = mybir.AluOpType.bypass

    nc = bacc.Bacc("TRN2", target_bir_lowering=False, debug=False,
                   num_devices=NCORES)

    def din(name, shape, dt=BF16):
        return nc.dram_tensor(name, shape, dt, kind="ExternalInput")

    xT_d = din("xT", [D_MODEL, TOK])
    encT_d = din("encT", [D_MODEL, TOK])
    xown_d = din("x_own", [TOWN, D_MODEL], F32)
    wq1_d = din("wq1", [128, KD * 128])
    wk1_d = din("wk1", [128, KD * 128])
    wv1_d = din("wv1", [128, KD * 128])
    bq1_d = din("bq1", [128, 1], F32)
    bk1_d = din("bk1", [128, 1], F32)
    bv1_d = din("bv1", [128, 1], F32)
    wo1_d = din("wo1", [128, KD * 1024])
    bo1_d = din("bo1", [128, 1024], F32)   # pre-broadcast
    wq2_d = din("wq2", [128, KD * KD * 128])
    bq2_d = din("bq2", [128, KD], F32)
    wk2_d = din("wk2", [128, KD * 128])
    wv2_d = din("wv2", [128, KD * 128])
    bk2_d = din("bk2", [128, 1], F32)
    bv2_d = din("bv2", [128, 1], F32)
    wo2_d = din("wo2", [128, KD * 1024])
    bo2_d = din("bo2", [128, 1024], F32)   # pre-broadcast
    w1_d = din("w1", [128, FC * KD * 128])
    b1_d = din("b1", [128, FC], F32)       # per-partition per-chunk
    w2_d = din("w2", [128, FC * 1024])
    b2_d = din("b2", [128, 1024], F32)     # pre-broadcast
    ident_d = din("ident", [128, 128])
    cm_d = din("cmask", [128, max(n_ctiles, 1) * 512])
    mb_d = din("mbias", [128, B * NBC], F32)
    bsel_d = din("bsel", [16, KD * 128])
    out_d = nc.dram_tensor("out", [TOWN, D_MODEL], F32, kind="ExternalOutput")

    CROSS_BLOCKS = {(t, c): 'full' for t in range(NBT) for c in range(NBC)}
    GROUPS = [list(range(NCORES))]

    with tile.TileContext(nc) as tc:
      with tc.tile_pool(name="const", bufs=1) as constp, \
           tc.tile_pool(name="fbuf", bufs=1) as fbuf, \
           tc.tile_pool(name="lnsmall", bufs=2) as lns, \
           tc.tile_pool(name="dram", bufs=1, space="DRAM") as dram, \
           tc.tile_pool(name="ps_aux", bufs=2, space="PSUM") as ps_aux:

        # ---- constants ----
        ones65 = constp.tile([1, 65], F32)
        nc.vector.memset(ones65[:], 1.0)
        ident = constp.tile([128, 128], BF16)
        nc.sync.dma_start(out=ident[:], in_=ident_d[:])
        cm = constp.tile([128, max(n_ctiles, 1) * 512], BF16)
        nc.sync.dma_start(out=cm[:], in_=cm_d[:])
        mb = constp.tile([128, B * NBC], F32)
        nc.sync.dma_start(out=mb[:], in_=mb_d[:])
        bsel = constp.tile([16, KD * 128], BF16)
        nc.sync.dma_start(out=bsel[:], in_=bsel_d[:])

        # ---- persistent activations ----
        out1 = fbuf.tile([128, 4 * 1024], F32, tag="out1")
        outT_a = fbuf.tile([128, KD * 512], BF16, tag="outT")  # out1T

        # ---- a2a dram buffers ----
        bar_in = dram.tile([NCORES, 16], BF16)
        bar_out = dram.tile([NCORES, 16], BF16)
        HT = TOWN // B   # 256 own tokens per batch half
        a2a1_in = [dram.tile([NCORES * 130, HT], BF16, name=f"a2a1i{b}")
                   for b in range(B)]
        a2a1_out = [dram.tile([NCORES * 130, HT], BF16, name=f"a2a1o{b}")
                    for b in range(B)]
        a2aq_in = [dram.tile([NCORES * 128, HT], BF16, name=f"a2aqi{b}")
                   for b in range(B)]
        a2aq_out = [dram.tile([NCORES * 128, HT], BF16, name=f"a2aqo{b}")
                    for b in range(B)]
        a2a2_in = [dram.tile([NCORES * 130, HT], BF16, name=f"a2a2i{b}")
                   for b in range(B)]
        a2a2_out = [dram.tile([NCORES * 130, HT], BF16, name=f"a2a2o{b}")
                    for b in range(B)]

        # startup barrier: absorb cross-core launch skew here (overlapped
        # with the initial input DMAs) instead of inside the first real a2a
        nc.sync.dma_start(out=bar_in[:], in_=ident[0:NCORES, 0:16])
        nc.gpsimd.collective_compute(
            "AllToAll", mybir.AluOpType.bypass, replica_groups=GROUPS,
            ins=[bar_in.opt()], outs=[bar_out.opt()])

        # ---------------- shared helpers ----------------
        def vaug_chunk_tr(vT_sb, vaug_sb, cg, key_mask=None):
            # PE-transpose V chunk cg ([128 (h,d), 128 tok] -> [128 tok,
            # (h,d)]) and scatter into vaug's per-head 65-column groups;
            # key_mask zeroes dropped keys (incl. the ones column).
            b, c = cg // NBC, cg % NBC
            ptr = ps_aux.tile([128, 128], BF16, tag="psaux", name="ptr")
            nc.tensor.transpose(ptr[:], vT_sb[:, 128 * cg:128 * (cg + 1)],
                                ident[:])
            dst = vaug_sb.rearrange("p (h b c d) -> p h b c d",
                                    h=HPC, b=B, c=NBC)
            nc.scalar.copy(dst[:, :, b, c, 0:64],
                           ptr.rearrange("p (h d) -> p h d", h=HPC))
            if key_mask is not None:
                for h in range(HPC):
                    base = 65 * (NBC * (B * h + b) + c)
                    sl = vaug_sb[:, base:base + 65]
                    nc.vector.tensor_scalar_mul(
                        sl, sl, key_mask[:, NBC * b + c:NBC * b + c + 1])

        def make_vaug(vT_sb, vaug_sb, key_mask=None):
            nc.vector.memset(vaug_sb[:], 1.0)
            for cg in range(TOK // 128):
                vaug_chunk_tr(vT_sb, vaug_sb, cg, key_mask)

        def vaug_slice(vaug_sb, h, b, c):
            base = 65 * (NBC * (B * h + b) + c)
            return vaug_sb[:, base:base + 65]

        EXPA = 0.125 * 1.4426950408889634 * 128.0   # Schraudolph mult
        EXPB = 16250.5                               # 127*128 - 5.5 centering

        def attention(pools, QTz, KT_sb, vaug_sb, stage_sb, blocks,
                      half_cb=None):
            # Software-pipelined attention, one ki-chunk per unit.
            # scores: full-128-row stationary (both heads' KT rows) x
            # zero-padded per-head QT -> both heads' scores in one 2-bank
            # psum; ONE exp per unit; attnV of the previous unit emitted
            # after this unit's scores; softmax division delayed one unit.
            ps_s, ps_o, ptp, smalls = pools
            units = []
            for b in range(B):
                for t in range(NBT):
                    clist = [c for c in range(NBC) if (t, c) in blocks]
                    for i, c in enumerate(clist):
                        units.append((b, t, c, i == 0, i == len(clist) - 1))

            po = {}          # live accumulation psums, per head
            pending = []     # [(unit, pts)] -- attnV delayed 2 units so the
                             # exps always complete well before the PE (in
                             # order) reaches the attnV that consumes them

            def emit_attnv(unit, pts):
                b, t, c, isfirst, islast = unit
                if isfirst:
                    for h in range(HPC):
                        po[h] = ps_o.tile([65, 512], F32, tag=f"po{h}",
                                          name=f"po{h}")
                kind = blocks[(t, c)]
                for h in range(HPC):
                    rhs = pts[h]
                    if kind != 'full':
                        idx = kind[1]
                        nc.vector.tensor_tensor(
                            rhs, rhs, cm[:, 512 * idx:512 * (idx + 1)],
                            op=MULT)
                    nc.tensor.matmul(
                        po[h][:], lhsT=vaug_slice(vaug_sb, h, b, c),
                        rhs=rhs, start=isfirst, stop=islast)
                if islast:
                    emit_division(b, t, dict(po))

            def emit_division(b, t, po_bt):
                # Ship the UNNORMALIZED output plus the denominator row to
                # the stage buffer; normalization happens post-a2a in the wo
                # phase (one approx-reciprocal + rank-1 broadcast per core).
                for h in range(HPC):
                    nc.scalar.copy(
                        stage_sb[:, h * TOK + 2048 * b + 512 * t:
                                 h * TOK + 2048 * b + 512 * t + 512],
                        po_bt[h][:])

            prev_b = 0
            for unit in units:
                b, t, c, isfirst, islast = unit
                if b != prev_b:
                    # flush the pipeline and hand batch 0's outputs to the
                    # caller (fires the first half-collective mid-attention)
                    for p_ in pending:
                        emit_attnv(*p_)
                    pending = []
                    if half_cb is not None:
                        half_cb()
                    prev_b = b
                ps = ps_s.tile([128, 1024], F32, tag="ps4", name="ps4")
                for h in range(HPC):
                    nc.tensor.matmul(
                        ps[:, 512 * h:512 * (h + 1)],
                        lhsT=KT_sb[:, 2048 * b + 128 * c:
                                   2048 * b + 128 * c + 128],
                        rhs=QTz[h][:, 2048 * b + 512 * t:
                                   2048 * b + 512 * t + 512],
                        start=True, stop=True)
                # softmax exp split across engines: h0 exact on ACT, h1
                # via the int16 Schraudolph bit-trick on the (otherwise
                # idle) DVE -- bf16 bits = int16(score*EXPA + EXPB).
                ptA = ptp.tile([128, 512], BF16, tag="ptA", name="ptA")
                nc.scalar.activation(ptA[:], ps[:, 0:512], EXP, scale=0.125)
                ptD = ptp.tile([128, 512], I16, tag="ptD", name="ptD")
                nc.vector.tensor_scalar(ptD[:], ps[:, 512:1024], EXPA, EXPB,
                                        op0=MULT, op1=ADD)
                if len(pending) >= 2:
                    emit_attnv(*pending.pop(0))
                pending.append((unit, (ptA[:], ptD[:].bitcast(BF16))))
            for p_ in pending:
                emit_attnv(*p_)

        def stage_to_a2a(stage_sb, a2a_in_t, b):
            # ship batch b's half: stage cols (h, b, 8 j-subtiles of 256)
            for h in range(HPC):
                o = a2a_in_t.rearrange("(j g r) s -> r j g s", j=NCORES,
                                       g=HPC)
                nc.sync.dma_start(
                    out=o[:, :, h],
                    in_=stage_sb.rearrange("r (h b j s) -> r h b j s",
                                           h=HPC, b=B, j=NCORES)[:, h, b])

        def a2a(in_t, out_t):
            nc.gpsimd.collective_compute(
                "AllToAll", mybir.AluOpType.bypass, replica_groups=GROUPS,
                ins=[in_t.opt()], outs=[out_t.opt()])

        def ln_one_m(pool, pres, m, outf_m):
            # Per-token-block layernorm: stats -> sqrt(var+eps) on ACT
            # (one sqrt table set per phase, no Ln/Exp set thrash) -> DVE
            # approximate reciprocal -> fused (x-mu)*rstd apply. Fully
            # pipelineable against the surrounding matmuls.
            bnst = pool.tile([128, 12], F32, tag="bnst")
            nc.vector.bn_stats(bnst[:, 0:6],
                               pres[:, 1024 * m:1024 * m + 512])
            nc.vector.bn_stats(bnst[:, 6:12],
                               pres[:, 1024 * m + 512:1024 * (m + 1)])
            stats = pool.tile([128, 2], F32, tag="stats")
            nc.vector.bn_aggr(stats[:], bnst[:])
            veps = pool.tile([128, 1], F32, tag="veps")
            nc.vector.tensor_scalar_add(veps[:], stats[:, 1:2], EPS)
            sd = pool.tile([128, 1], F32, tag="sd")
            nc.scalar.activation(sd[:], veps[:], SQRTF)
            rstd = pool.tile([128, 1], F32, tag="rstd")
            nc.vector.reciprocal_approx_fast(out=rstd[:], in_=sd[:])
            nc.vector.tensor_scalar(
                outf_m, pres[:, 1024 * m:1024 * (m + 1)],
                stats[:, 0:1], rstd[:], op0=SUB, op1=MULT)

        def normalize_at(pool, at_sb, a2a_out_t, hb):
            # dn[h, s] = denominator of head h for own-token s (half hb)
            dn = pool.tile([16, HT], BF16, tag=f"dn{hb}", name=f"dn{hb}")
            for g in range(HPC):
                nc.sync.dma_start(
                    out=dn.rearrange("(dc g) s -> g dc s", g=HPC)[g],
                    in_=a2a_out_t.rearrange("(dc g r) s -> g r dc s",
                                            dc=KD, g=HPC)[g, 64])
            dnf = pool.tile([16, HT], F32, tag=f"dnf{hb}", name=f"dnf{hb}")
            nc.vector.tensor_copy(dnf[:], dn[:])
            dnr = pool.tile([16, HT], F32, tag=f"dnr{hb}", name=f"dnr{hb}")
            nc.vector.reciprocal_approx_fast(out=dnr[:], in_=dnf[:])
            dnb = pool.tile([16, HT], BF16, tag=f"dnb{hb}", name=f"dnb{hb}")
            nc.vector.tensor_copy(dnb[:], dnr[:])
            atv = at_sb.rearrange("p (dc b s) -> p dc b s", dc=KD, b=B)
            for dc in range(KD):
                rb = ps_aux.tile([128, HT], F32, tag="psaux", name="rb")
                nc.tensor.matmul(rb[:], lhsT=bsel[:, 128 * dc:128 * (dc + 1)],
                                 rhs=dnb[:], start=True, stop=True)
                nc.vector.tensor_tensor(
                    atv[:, dc, hb], atv[:, dc, hb], rb[:], op=MULT)

        def wo_ln_block(pool1, pool, pstr, at_sb, wo_sb, bo_sb, resid_of,
                        outf, outT_sb, mid_cb=None):
            # outf[:, m*1024: ...] = LN(resid + at^T @ wo + bo), per m-tile.
            # bo_sb is pre-broadcast [128, 1024]; the output transpose runs
            # on the (otherwise idle) PE via the identity trick rather than
            # the serializing DMA-transpose path. LN is batched over all 4
            # m-tiles to avoid ACT table-set thrash.
            pres = pool1.tile([128, 4 * 1024], F32, tag="pres")
            inplace = outf is None
            if inplace:
                outf = pres
            for m in range(4):
                if m == 2 and mid_cb is not None:
                    mid_cb()
                resid = resid_of(m)
                for eh in range(2):
                    pw = ps_aux.tile([128, 512], F32, tag="psaux", name="pw")
                    for dc in range(KD):
                        nc.tensor.matmul(
                            pw[:],
                            lhsT=at_sb[:, 512 * dc + 128 * m:
                                       512 * dc + 128 * m + 128],
                            rhs=wo_sb[:, 1024 * dc + 512 * eh:
                                      1024 * dc + 512 * eh + 512],
                            start=(dc == 0), stop=(dc == KD - 1))
                    nc.vector.tensor_tensor(
                        pres[:, 1024 * m + 512 * eh:
                             1024 * m + 512 * (eh + 1)], pw[:],
                        resid[:, 512 * eh:512 * (eh + 1)], op=ADD)
                    nc.vector.tensor_tensor(
                        pres[:, 1024 * m + 512 * eh:
                             1024 * m + 512 * (eh + 1)],
                        pres[:, 1024 * m + 512 * eh:
                             1024 * m + 512 * (eh + 1)],
                        bo_sb[:, 512 * eh:512 * (eh + 1)], op=ADD)
            outfv = outf.rearrange("p (m e) -> p m e", m=4)
            for m in range(4):
                ln_one_m(pool, pres, m, outfv[:, m])
                # bf16 copy + PE transpose into outT
                obf = pool.tile([128, 1024], BF16, tag="obf")
                nc.scalar.copy(obf[:], outfv[:, m])
                for j in range(KD):
                    ptr = pstr.tile([128, 128], BF16, tag="ptr")
                    nc.tensor.transpose(ptr[:], obf[:, 128 * j:128 * (j + 1)],
                                        ident[:])
                    nc.vector.tensor_copy(
                        outT_sb[:, 512 * j + 128 * m:512 * j + 128 * m + 128],
                        ptr[:])
            return outf

        # p3keep: cross-attn K/V/Q buffers that must survive into attn2
        with tc.tile_pool(name="p3keep", bufs=1) as p3k:
            KT2 = p3k.tile([128, TOK], BF16, tag="KT2")
            vaug2 = p3k.tile([128, HPC * B * NBC * 65], BF16, tag="vaug2")
            vT2 = p3k.tile([128, TOK], BF16, tag="vT2")
            wk2 = p3k.tile([128, KD * 128], BF16, tag="wk2")
            wv2 = p3k.tile([128, KD * 128], BF16, tag="wv2")
            bk2 = p3k.tile([128, 1], F32, tag="bk2")
            bv2 = p3k.tile([128, 1], F32, tag="bv2")

            # ============= phases 1-2: self attention =====================
            pe3s = tc.alloc_tile_pool(name="pencT", bufs=3)
            enc_pre = []

            def prefetch_enc(n):
                for j in range(len(enc_pre), n):
                    et = pe3s.tile([128, KD * 512], BF16, tag="et",
                                   name=f"etp{j}")
                    nc.sync.dma_start(
                        out=et.rearrange("p (k s) -> p k s", k=KD),
                        in_=encT_d.rearrange(
                            "(k p) t -> p k t",
                            p=128)[:, :, 512 * j:512 * (j + 1)])
                    enc_pre.append(et)
            with tc.tile_pool(name="pact1", bufs=1) as pact1, \
                 tc.tile_pool(name="p12s", bufs=2) as p12s, \
                 tc.tile_pool(name="pt12", bufs=4) as pt12:
                KT = pact1.tile([128, TOK], BF16, tag="KT")
                # vT shares the (larger) stage slot — it dies before stage1
                # is written.
                vT1 = pact1.tile([128, TOK], BF16, tag="stage", name="vT1")
                vaug1 = pact1.tile([128, HPC * B * NBC * 65], BF16,
                                   tag="vaug")
                QTz0 = pact1.tile([128, TOK], BF16, tag="QTz0")
                QTz1 = pact1.tile([128, TOK], BF16, tag="QTz1")
                nc.vector.memset(QTz0[64:128, :], 0.0)
                nc.vector.memset(QTz1[0:64, :], 0.0)

                with tc.tile_pool(name="pxw", bufs=1) as pxw, \
                     tc.tile_pool(name="pxs", bufs=2) as pxs:
                    wq1 = pxw.tile([128, KD * 128], BF16, tag="wq1")
                    wk1 = pxw.tile([128, KD * 128], BF16, tag="wk1")
                    wv1 = pxw.tile([128, KD * 128], BF16, tag="wv1")
                    nc.sync.dma_start(out=wq1[:], in_=wq1_d[:])
                    nc.sync.dma_start(out=wk1[:], in_=wk1_d[:])
                    nc.sync.dma_start(out=wv1[:], in_=wv1_d[:])
                    bq1 = pxw.tile([128, 1], F32, tag="bq1")
                    bk1 = pxw.tile([128, 1], F32, tag="bk1")
                    bv1 = pxw.tile([128, 1], F32, tag="bv1")
                    nc.sync.dma_start(out=bq1[:], in_=bq1_d[:])
                    nc.sync.dma_start(out=bk1[:], in_=bk1_d[:])
                    nc.sync.dma_start(out=bv1[:], in_=bv1_d[:])
                    nc.sync.dma_start(out=bk2[:], in_=bk2_d[:])
                    nc.sync.dma_start(out=bv2[:], in_=bv2_d[:])
                    nc.sync.dma_start(out=wk2[:], in_=wk2_d[:])
                    nc.sync.dma_start(out=wv2[:], in_=wv2_d[:])
                    nc.vector.memset(vaug1[:], 1.0)

                    def to_qtz(ps, j):
                        nc.vector.tensor_scalar_add(
                            QTz0[0:64, 512 * j:512 * (j + 1)], ps[0:64, :],
                            bq1[0:64, :])
                        nc.vector.tensor_scalar_add(
                            QTz1[64:128, 512 * j:512 * (j + 1)],
                            ps[64:128, :], bq1[64:128, :])

                    # xT streamed per 512-token stripe: the v/q/k projections
                    # for one stripe start after ~1/8 of the input load
                    xT_dv = xT_d.rearrange("(k p) t -> p k t", p=128)
                    for j in range(TOK // 512):
                        xs = pxs.tile([128, KD * 512], BF16, tag="xs")
                        xsv = xs.rearrange("p (k s) -> p k s", k=KD)
                        nc.sync.dma_start(
                            out=xsv[:],
                            in_=xT_dv[:, :, 512 * j:512 * (j + 1)])

                        def proj_t(dst_of):
                            w_sb = dst_of[0]
                            ps = ps_aux.tile([128, 512], F32, tag="psaux",
                                             name="psp")
                            for k in range(KD):
                                nc.tensor.matmul(
                                    ps[:],
                                    lhsT=w_sb[:, 128 * k:128 * (k + 1)],
                                    rhs=xsv[:, k], start=(k == 0),
                                    stop=(k == KD - 1))
                            dst_of[1](ps)

                        def to_full(dst, bias):
                            def f(ps):
                                nc.vector.tensor_scalar_add(
                                    dst[:, 512 * j:512 * (j + 1)], ps[:],
                                    bias[:])
                            return f

                        proj_t((wv1, to_full(vT1, bv1)))
                        for cg in range(4 * j, 4 * j + 4):
                            vaug_chunk_tr(vT1, vaug1, cg)
                        proj_t((wq1, lambda ps: to_qtz(ps, j)))
                        proj_t((wk1, to_full(KT, bk1)))
                # pxw/pxs closed
                prefetch_enc(2)

                stage1 = pact1.tile([65, HPC * TOK], BF16, tag="stage",
                                    name="stage1")

                def attn1_half():
                    stage_to_a2a(stage1, a2a1_in[0], 0)
                    a2a(a2a1_in[0], a2a1_out[0])

                with tc.tile_pool(name="ps_s1", bufs=2,
                                  space="PSUM") as ps_s1, \
                     tc.tile_pool(name="ps_o1", bufs=1,
                                  space="PSUM") as ps_o1:
                    attention((ps_s1, ps_o1, pt12, p12s),
                              (QTz0, QTz1), KT, vaug1, stage1, self_blocks,
                              half_cb=attn1_half)
                stage_to_a2a(stage1, a2a1_in[1], 1)
            # pact1 closed (attn1 buffers free)

            a2a(a2a1_in[1], a2a1_out[1])

            # ===== phase 3: cross K/V projection =========================
            # encT streamed per 512-token stripe (first two prefetched
            # during attn1); the first half is emitted between wo1's two
            # m-halves (fills the a2a1B wait), the rest after the a2aq
            # doorbells (fills that gap).
            def emit_cross_stripes(j0, j1):
                for j in range(j0, j1):
                    if j < len(enc_pre):
                        et = enc_pre[j]
                    else:
                        et = pe3s.tile([128, KD * 512], BF16, tag="et",
                                       name=f"et{j}")
                        nc.sync.dma_start(
                            out=et.rearrange("p (k s) -> p k s", k=KD),
                            in_=encT_d.rearrange(
                                "(k p) t -> p k t",
                                p=128)[:, :, 512 * j:512 * (j + 1)])
                    etv = et.rearrange("p (k s) -> p k s", k=KD)
                    for w_sb, bias, dst in ((wv2, bv2, vT2), (wk2, bk2, KT2)):
                        ps = ps_aux.tile([128, 512], F32, tag="psaux",
                                         name="psc")
                        for k in range(KD):
                            nc.tensor.matmul(
                                ps[:],
                                lhsT=w_sb[:, 128 * k:128 * (k + 1)],
                                rhs=etv[:, k], start=(k == 0),
                                stop=(k == KD - 1))
                        nc.vector.tensor_scalar_add(
                            dst[:, 512 * j:512 * (j + 1)], ps[:], bias[:])

            # ===== phase 4: wo1 + residual + LN1 + transpose ===============
            with tc.tile_pool(name="p4", bufs=1) as p4, \
                 tc.tile_pool(name="p4s", bufs=2) as p4s:
                at1 = p4.tile([128, KD * 512], BF16, tag="at1")

                def load_at_half(at_sb, srcs, hb):
                    for g in range(HPC):
                        nc.sync.dma_start(
                            out=at_sb.rearrange("p (dc b s) -> p dc b s",
                                                dc=KD, b=B)[
                                64 * g:64 * (g + 1), :, hb],
                            in_=srcs[hb].rearrange(
                                "(dc g r) s -> g r dc s",
                                dc=KD, g=HPC)[g, 0:64])

                load_at_half(at1, a2a1_out, 0)
                normalize_at(p4, at1, a2a1_out[0], 0)
                wo1 = p4.tile([128, KD * 1024], BF16, tag="wo1")
                nc.sync.dma_start(out=wo1[:], in_=wo1_d[:])
                bo1 = p4.tile([128, 1024], F32, tag="bo1")
                nc.sync.dma_start(out=bo1[:], in_=bo1_d[:])
                xown = p4.tile([128, 4 * 1024], F32, tag="xown")
                nc.sync.dma_start(
                    out=xown.rearrange("p (m e) -> p m e", m=4),
                    in_=xown_d.rearrange("(m p) e -> p m e", p=128))
                xownv = xown.rearrange("p (m e) -> p m e", m=4)

                with tc.tile_pool(name="ps_tr4", bufs=2,
                                  space="PSUM") as ps_tr4:
                    def wo1_mid():
                        load_at_half(at1, a2a1_out, 1)
                        emit_cross_stripes(0, 3)
                        normalize_at(p4, at1, a2a1_out[1], 1)

                    wo_ln_block(p4, p4s, ps_tr4, at1, wo1, bo1,
                                lambda m: xownv[:, m], out1, outT_a,
                                mid_cb=wo1_mid)

            emit_cross_stripes(3, 5)

            # ===== phase 5: cross q projection + a2a ========================
            with tc.tile_pool(name="p5", bufs=1) as p5:
                wq2 = p5.tile([128, KD * KD * 128], BF16, tag="wq2")
                nc.sync.dma_start(out=wq2[:], in_=wq2_d[:])
                bq2 = p5.tile([128, KD], F32, tag="bq2")
                nc.sync.dma_start(out=bq2[:], in_=bq2_d[:])
                qt2 = p5.tile([128, KD * 512], BF16, tag="qt2")
                for j in range(KD):
                    pq = ps_aux.tile([128, 512], F32, tag="psaux",
                                     name="pq")
                    for k in range(KD):
                        nc.tensor.matmul(
                            pq[:],
                            lhsT=wq2[:, 1024 * j + 128 * k:
                                     1024 * j + 128 * k + 128],
                            rhs=outT_a[:, 512 * k:512 * (k + 1)],
                            start=(k == 0), stop=(k == KD - 1))
                    nc.vector.tensor_scalar_add(
                        qt2[:, 512 * j:512 * (j + 1)], pq[:],
                        bq2[:, j:j + 1])
                for hb in range(B):
                    nc.sync.dma_start(
                        out=a2aq_in[hb].rearrange("(j p) s -> p j s", p=128),
                        in_=qt2.rearrange("p (j b s) -> p j b s", j=KD,
                                          b=B)[:, :, hb])
            a2a(a2aq_in[0], a2aq_out[0])
            a2a(a2aq_in[1], a2aq_out[1])

            emit_cross_stripes(5, 8)
            pe3s.release()
            # vaug2 build follows the cross projections (needs only vT2)
            make_vaug(vT2, vaug2, key_mask=mb)

            QT2z0 = p3k.tile([128, TOK], BF16, tag="qt2z0", name="QT2z0")
            QT2z1 = p3k.tile([128, TOK], BF16, tag="qt2z1", name="QT2z1")
            nc.vector.memset(QT2z0[64:128, :], 0.0)
            nc.vector.memset(QT2z1[0:64, :], 0.0)
            for hb in range(B):
                aqv = a2aq_out[hb].rearrange("(i p) s -> p i s", p=128)
                z0v = QT2z0.rearrange("p (b i s) -> p b i s", b=B,
                                      i=NCORES)
                z1v = QT2z1.rearrange("p (b i s) -> p b i s", b=B,
                                      i=NCORES)
                nc.sync.dma_start(out=z0v[0:64, hb], in_=aqv[0:64])
                nc.sync.dma_start(out=z1v[64:128, hb], in_=aqv[64:128])

            # ===== phase 6: cross attention -> a2a2 =========================
            with tc.tile_pool(name="p6", bufs=1) as p6, \
                 tc.tile_pool(name="p6s", bufs=2) as p6s, \
                 tc.tile_pool(name="pt6", bufs=4) as pt6, \
                 tc.tile_pool(name="ps_s2", bufs=2, space="PSUM") as ps_s2, \
                 tc.tile_pool(name="ps_o2", bufs=1, space="PSUM") as ps_o2:
                stage2 = p6.tile([65, HPC * TOK], BF16, tag="stage2")

                def attn2_half():
                    stage_to_a2a(stage2, a2a2_in[0], 0)
                    a2a(a2a2_in[0], a2a2_out[0])

                attention((ps_s2, ps_o2, pt6, p6s),
                          (QT2z0, QT2z1), KT2, vaug2, stage2, CROSS_BLOCKS,
                          half_cb=attn2_half)
                stage_to_a2a(stage2, a2a2_in[1], 1)
            a2a(a2a2_in[1], a2a2_out[1])

        # ============ phases 7-8: wo2 + LN2 + FFN + LN3 =====================
        with tc.tile_pool(name="p78", bufs=1) as p78, \
             tc.tile_pool(name="p78s", bufs=2) as p78s, \
             tc.tile_pool(name="w1stream", bufs=3) as w1s_pool:
            at2 = p78.tile([128, KD * 512], BF16, tag="at2")

            def load_at2_half(hb):
                for g in range(HPC):
                    nc.sync.dma_start(
                        out=at2.rearrange("p (dc b s) -> p dc b s",
                                          dc=KD, b=B)[
                            64 * g:64 * (g + 1), :, hb],
                        in_=a2a2_out[hb].rearrange(
                            "(dc g r) s -> g r dc s",
                            dc=KD, g=HPC)[g, 0:64])

            load_at2_half(0)
            normalize_at(p78, at2, a2a2_out[0], 0)
            wo2 = p78.tile([128, KD * 1024], BF16, tag="wo2")
            nc.sync.dma_start(out=wo2[:], in_=wo2_d[:])
            bo2 = p78.tile([128, 1024], F32, tag="bo2")
            nc.sync.dma_start(out=bo2[:], in_=bo2_d[:])
            b1 = p78.tile([128, FC], F32, tag="b1")
            nc.sync.dma_start(out=b1[:], in_=b1_d[:])
            b2 = p78.tile([128, 1024], F32, tag="b2")
            nc.sync.dma_start(out=b2[:], in_=b2_d[:])
            w2 = p78.tile([128, FC * 1024], BF16, tag="w2")

            # out2T reuses the out1T slot (out1T dead after phase 5)
            outT_b = fbuf.tile([128, KD * 512], BF16, tag="outT",
                               name="outT_b")
            out1v = out1.rearrange("p (m e) -> p m e", m=4)
            with tc.tile_pool(name="ps_tr78", bufs=2,
                              space="PSUM") as ps_tr78:
                def wo2_mid():
                    load_at2_half(1)
                    # big FFN w2 weight load AFTER the collective's data is
                    # in flight so the 8MB stream doesn't fight the a2a for
                    # HBM bandwidth
                    nc.sync.dma_start(out=w2[:], in_=w2_d[:])
                    normalize_at(p78, at2, a2a2_out[1], 1)

                out2 = wo_ln_block(p78, p78s, ps_tr78, at2, wo2, bo2,
                                   lambda m: out1v[:, m], None, outT_b,
                                   mid_cb=wo2_mid)

            hT = p78.tile([128, FC * 512], BF16, tag="hT")
            for fc in range(FC):
                w1t = w1s_pool.tile([128, KD * 128], BF16, tag="w1s")
                nc.sync.dma_start(out=w1t[:],
                                  in_=w1_d[:, 1024 * fc:1024 * (fc + 1)])
                ph = ps_aux.tile([128, 512], F32, tag="psaux", name="ph")
                for k in range(KD):
                    nc.tensor.matmul(ph[:],
                                     lhsT=w1t[:, 128 * k:128 * (k + 1)],
                                     rhs=outT_b[:, 512 * k:512 * (k + 1)],
                                     start=(k == 0), stop=(k == KD - 1))
                nc.vector.tensor_scalar(hT[:, 512 * fc:512 * (fc + 1)],
                                        ph[:], b1[:, fc:fc + 1], 0.0,
                                        op0=ADD, op1=mybir.AluOpType.max)

            out2v = out2.rearrange("p (m e) -> p m e", m=4)
            pres2 = p78.tile([128, 4 * 1024], F32, tag="pres2")
            with tc.tile_pool(name="ps_w2", bufs=2, space="PSUM") as ps_w2:
                for m in range(4):
                    # eh inner: both 512-col halves share each hT stationary
                    py = ps_w2.tile([128, 1024], F32, tag="py", name="py")
                    for fc in range(FC):
                        for eh in range(2):
                            nc.tensor.matmul(
                                py[:, 512 * eh:512 * (eh + 1)],
                                lhsT=hT[:, 512 * fc + 128 * m:
                                        512 * fc + 128 * m + 128],
                                rhs=w2[:, 1024 * fc + 512 * eh:
                                       1024 * fc + 512 * eh + 512],
                                start=(fc == 0), stop=(fc == FC - 1))
                    for eh in range(2):
                        nc.vector.tensor_tensor(
                            pres2[:, 1024 * m + 512 * eh:
                                  1024 * m + 512 * (eh + 1)],
                            py[:, 512 * eh:512 * (eh + 1)],
                            out2v[:, m, 512 * eh:512 * (eh + 1)], op=ADD)
                        nc.vector.tensor_tensor(
                            pres2[:, 1024 * m + 512 * eh:
                                  1024 * m + 512 * (eh + 1)],
                            pres2[:, 1024 * m + 512 * eh:
                                  1024 * m + 512 * (eh + 1)],
                            b2[:, 512 * eh:512 * (eh + 1)], op=ADD)

            # final LN applied in-place into pres2, then DMA'd out, per m
            pres2v = pres2.rearrange("p (m e) -> p m e", m=4)
            for m in range(4):
                ln_one_m(p78s, pres2, m, pres2v[:, m])
                nc.sync.dma_start(out=out_d[128 * m:128 * (m + 1), :],
                                  in_=pres2v[:, m])

    nc.compile()
    return nc


def _to_bf(a):
    return np.ascontiguousarray(np.asarray(a, np.float32).astype(BF))


def _rechunk_k(w):
    """[K*128, M] -> [128, K*M] with col k*M + m = w[k*128 + p, m]."""
    K = w.shape[0] // 128
    M = w.shape[1]
    return np.ascontiguousarray(
        w.reshape(K, 128, M).transpose(1, 0, 2).reshape(128, K * M))


def _analyze_self_mask(mask):
    """mask [S, S] (1 = disallowed), orientation [q, k].

    Returns blocks dict (t, c) -> 'full' | ('tile', idx), list of unique
    multiplicative tiles [128, 512] (bf16), for a block grid over one batch.
    Blocks where everything is disallowed are omitted.
    """
    add = np.float32(-1e9) * np.asarray(mask, np.float32)
    mult = np.exp(add.T)  # [k, q] multiplicative
    blocks = {}
    tiles = []
    tile_ids = {}
    for t in range(NBT):
        for c in range(NBC):
            sub = mult[128 * c:128 * (c + 1), 512 * t:512 * (t + 1)]
            if not sub.any():
                continue
            if (sub == 1.0).all():
                blocks[(t, c)] = 'full'
                continue
            key = sub.tobytes()
            if key not in tile_ids:
                tile_ids[key] = len(tiles)
                tiles.append(sub.astype(BF))
            blocks[(t, c)] = ('tile', tile_ids[key])
    return blocks, tiles


def kernel(**inputs):
    from concourse.bass_utils import run_bass_kernel_spmd

    x = np.asarray(inputs["x"], np.float32)
    enc = np.asarray(inputs["enc_output"], np.float32)
    lam = np.asarray(inputs["look_ahead_mask"], np.float32)[0, 0]
    pad = np.asarray(inputs["padding_mask"], np.float32)  # [B,1,1,S]

    self_blocks, ctiles = _analyze_self_mask(lam)
    n_ctiles = len(ctiles)
    key = (tuple(sorted(self_blocks.items())), n_ctiles)
    if key not in _PROG_CACHE:
        _PROG_CACHE[key] = _build_program(self_blocks, n_ctiles)
    nc = _PROG_CACHE[key]

    # ---- shared (core-independent) host prep ----
    xf = x.reshape(TOK, D_MODEL)             # flattened batch-major tokens
    encf = enc.reshape(TOK, D_MODEL)
    xT = _to_bf(xf.T)                        # [1024, 4096]
    encT = _to_bf(encf.T)
    if n_ctiles:
        cmask = np.concatenate(ctiles, axis=1)
    else:
        cmask = np.zeros((128, 512), BF)
    cmask = np.ascontiguousarray(cmask)
    # cross-attn key-keep mask per enc token: [128, B*16], col b*16+c
    mb = np.exp(np.float32(-1e9) * pad[:, 0, 0, :]).reshape(B, NBC, 128)
    mb = np.ascontiguousarray(mb.transpose(2, 0, 1).reshape(128, B * NBC)
                              ).astype(np.float32)

    w1f = np.asarray(inputs["ffn_w1"], np.float32)
    # w1 stationary layout: [128, fc*1024 + k*128 + m] = w1[k*128+p, fc*128+m]
    w1r = w1f.reshape(KD, 128, FC, 128).transpose(1, 2, 0, 3)
    w1r = _to_bf(w1r.reshape(128, FC * KD * 128))
    w2r = _to_bf(_rechunk_k(np.asarray(inputs["ffn_w2"], np.float32)))
    # b1 per-partition per-chunk [128, FC]; b2/bo pre-broadcast [128, 1024]
    b1 = np.ascontiguousarray(
        np.asarray(inputs["ffn_b1"], np.float32).reshape(FC, 128).T)
    b2 = np.ascontiguousarray(np.broadcast_to(
        np.asarray(inputs["ffn_b2"], np.float32)[None, :], (128, 1024)))

    wo1r = _to_bf(_rechunk_k(np.asarray(inputs["mha1_wo"], np.float32)))
    wo2r = _to_bf(_rechunk_k(np.asarray(inputs["mha2_wo"], np.float32)))
    bo1 = np.ascontiguousarray(np.broadcast_to(
        np.asarray(inputs["mha1_bo"], np.float32)[None, :], (128, 1024)))
    bo2 = np.ascontiguousarray(np.broadcast_to(
        np.asarray(inputs["mha2_bo"], np.float32)[None, :], (128, 1024)))
    ident = np.eye(128, dtype=np.float32).astype(BF)
    # bsel[h, dc*128 + p] = 1 where head h owns feature row p of chunk dc
    bsel = np.zeros((16, KD * 128), np.float32)
    for dc in range(KD):
        for p in range(128):
            bsel[2 * dc + p // 64, 128 * dc + p] = 1.0
    bsel = bsel.astype(BF)

    wq2_full = np.asarray(inputs["mha2_wq"], np.float32)
    # wq2 stationary layout: [128, j*1024 + k*128 + m] = wq2[k*128+p, j*128+m]
    wq2r = wq2_full.reshape(KD, 128, KD, 128).transpose(1, 2, 0, 3)
    wq2r = _to_bf(wq2r.reshape(128, KD * KD * 128))
    bq2 = np.asarray(inputs["mha2_bq"], np.float32).reshape(KD, 128)
    bq2 = np.ascontiguousarray(bq2.T).astype(np.float32)  # [128, KD]

    in_maps = []
    for j in range(NCORES):
        hs = slice(128 * j, 128 * (j + 1))       # this core's 2 heads' cols
        m = {
            "xT": xT, "encT": encT,
            "x_own": np.ascontiguousarray(np.concatenate(
                [xf[256 * j:256 * (j + 1)],
                 xf[2048 + 256 * j:2048 + 256 * (j + 1)]], axis=0)),
            "cmask": cmask, "mbias": mb,
            "w1": w1r, "b1": b1, "w2": w2r, "b2": b2,
            "wo1": wo1r, "bo1": bo1, "wo2": wo2r, "bo2": bo2,
            "wq2": wq2r, "bq2": bq2, "ident": ident, "bsel": bsel,
        }
        for pre, name in (("wq1", "mha1_wq"), ("wk1", "mha1_wk"),
                          ("wv1", "mha1_wv"), ("wk2", "mha2_wk"),
                          ("wv2", "mha2_wv")):
            w = np.asarray(inputs[name], np.float32)[:, hs]
            m[pre] = _to_bf(_rechunk_k(w))
        for pre, name in (("bq1", "mha1_bq"), ("bk1", "mha1_bk"),
                          ("bv1", "mha1_bv"), ("bk2", "mha2_bk"),
                          ("bv2", "mha2_bv")):
            bvec = np.asarray(inputs[name], np.float32)[hs]
            m[pre] = np.ascontiguousarray(bvec[:, None])
        in_maps.append(m)

    res = run_bass_kernel_spmd(nc, in_maps, list(range(NCORES)))
    out = np.empty((TOK, D_MODEL), np.float32)
    for j in range(NCORES):
        r = res.results[j]["out"]
        out[256 * j:256 * (j + 1)] = r[0:256]
        out[2048 + 256 * j:2048 + 256 * (j + 1)] = r[256:512]
    return out.reshape(B, S, D_MODEL)
